# revision 28
# baseline (speedup 1.0000x reference)
"""CapsuleLayer (dynamic routing, 3 iterations) on 8 Trainium2 NeuronCores.

V2 structure — replicated iteration 1, i-sharded iterations 2-3:
  - Iteration 1 uses uniform c = 1/10 (softmax of zeros), so
    s1 = 0.1 * (xt_full.T @ Wl_full) needs no routing state. Instead of
    computing an i-shard partial and paying an AllGather round (the
    collective subsystem is still booting until ~77us anyway), EVERY core
    computes the FULL s1 itself: 72 chunk matmuls on the otherwise-idle
    PE, fed by a full (replicated) bf16 copy of xt and Wl (~8MB DMA).
    This removes AG1 and its ~35us round trip entirely.
  - Iterations 2-3 are i-sharded exactly as V1: 144 capsules per core,
    1152 rows = 9 chunks; s partials exchanged via one fp8-e4m3
    AllGather + on-chip tree reduce (iteration 2) and one f32
    ReduceScatter (iteration 3, batch-sharded output).
  - b_ij update via the Gram trick: Q = xik.T @ v (PE), p = Wl*Q (DVE),
    uv = F.T @ p with F = kron(I16, ones8x8)/B (PE), keeping b
    row-replicated over k.
  - A tiny warm-up AllGather at kernel start overlaps the one-time
    ncfw/collective boot (~68us) with input DMA + the s1 compute.
  - sqrt via int bit-trick + Newton on the DVE (no Sqrt/Ln ACT tables).
  - Routing matmuls in bf16 (fp32 PE matmuls lower 8x slower); the
    2e-2 gate leaves plenty of room (measured ~3e-3).
"""
import sys

if "/opt/trn_rl_repo" not in sys.path:
    sys.path.insert(0, "/opt/trn_rl_repo")

import numpy as np

import os
N_CORES = int(os.environ.get("KERNEL_CORES", "8"))
B, IN_SIZE, I_TOT = 256, 8, 1152
N_NODE, O_SZ = 10, 16
NO = N_NODE * O_SZ          # 160
I_SH = I_TOT // N_CORES     # 144 capsules per core
JR = I_SH * IN_SIZE         # 1152 local rows per core
NCH = JR // 128             # 9 local contraction chunks
JF = I_TOT * IN_SIZE        # 9216 full rows
NCF = JF // 128             # 72 full contraction chunks
BC = B // 128               # 2 batch chunks
B_SH = B // N_CORES         # 32 batch rows per core after ReduceScatter

RSQRT_MAGIC = 0x5F3759DF

_CACHE = {}


def _build_program():
    import concourse.bacc as bacc
    import concourse.tile as tile
    import concourse.mybir as mybir

    f32 = mybir.dt.float32
    bf16 = mybir.dt.bfloat16
    f8 = mybir.dt.float8e4
    i32 = mybir.dt.int32
    AF = mybir.ActivationFunctionType
    ALU = mybir.AluOpType
    AX = mybir.AxisListType

    nc = bacc.Bacc("TRN2", target_bir_lowering=False, debug=False,
                   enable_asserts=True, num_devices=N_CORES)

    xtf_d = nc.dram_tensor("xtf", [JF, B], bf16, kind="ExternalInput").ap()
    wlf_d = nc.dram_tensor("wlf", [JF, NO], bf16, kind="ExternalInput").ap()
    xt_d = nc.dram_tensor("xt", [JR, B], bf16, kind="ExternalInput").ap()
    xt8_d = nc.dram_tensor("xt8", [JR, B], f8, kind="ExternalInput").ap()
    xik_d = nc.dram_tensor("xik", [B, JR], f8, kind="ExternalInput").ap()
    wl_d = nc.dram_tensor("wl", [JR, NO], bf16, kind="ExternalInput").ap()
    f_d = nc.dram_tensor("fmat", [128, 128], bf16, kind="ExternalInput").ap()
    y_d = nc.dram_tensor("y", [B_SH, NO], f32, kind="ExternalOutput").ap()

    RG = [list(range(N_CORES))]

    with tile.TileContext(nc) as tc:
        with tc.tile_pool(name="persist", bufs=1) as pp, \
             tc.tile_pool(name="work", bufs=1) as wp, \
             tc.tile_pool(name="ps_s", bufs=2, space="PSUM") as ps_s, \
             tc.tile_pool(name="ps_q", bufs=3, space="PSUM") as ps_q, \
             tc.tile_pool(name="ps_f", bufs=1, space="PSUM") as ps_f, \
             tc.tile_pool(name="dram", bufs=1, space="DRAM") as dp:

            # ---------------- input loads ----------------
            xtf_sb = pp.tile([128, NCF, B], bf16, name="xtf_sb", tag="xtf_sb")
            wlf_sb = pp.tile([128, NCF, NO], bf16, name="wlf_sb",
                             tag="wlf_sb")
            xt_sb = pp.tile([128, NCH, B], bf16, name="xt_sb", tag="xt_sb")
            xt8_sb = pp.tile([128, NCH, B], f8, name="xt8_sb", tag="xt8_sb")
            xik_sb = pp.tile([128, BC, JR], f8, name="xik_sb", tag="xik_sb")
            wl_sb = pp.tile([128, NCH, NO], bf16, name="wl_sb", tag="wl_sb")
            f_sb = pp.tile([128, 128], bf16, name="f_sb", tag="f_sb")
            b_sb = pp.tile([128, NCH, N_NODE], f32, name="b_sb", tag="b_sb")

            # Warm-up collective: starts the one-time ncfw/TOPSP collective
            # boot (~68us) at program start, overlapping the input DMAs and
            # the replicated s1 compute.
            warm_in = dp.tile([128, 4], bf16, name="warm_in", tag="warm_in")
            warm_out = dp.tile([N_CORES * 128, 4], bf16, name="warm_out",
                               tag="warm_out")
            nc.gpsimd.collective_compute(
                "AllGather", ALU.bypass, replica_groups=RG,
                ins=[warm_in.opt()], outs=[warm_out.opt()])

            # Full xt/Wl stream in 12-chunk groups round-robined over the
            # three DMA-capable engine queues, in s1 consumption order.
            engs = [nc.sync, nc.scalar, nc.gpsimd]
            xtf3 = xtf_d.rearrange("(c p) b -> p c b", p=128)
            wlf3 = wlf_d.rearrange("(c p) f -> p c f", p=128)
            GW = 12
            for g in range(NCF // GW):
                eng = engs[g % 3]
                sl = slice(g * GW, (g + 1) * GW)
                eng.dma_start(xtf_sb[:, sl, :], xtf3[:, sl, :])
                eng.dma_start(wlf_sb[:, sl, :], wlf3[:, sl, :])
            # Local tensors (needed from the iter-1 b-update onward).
            xt3 = xt_d.rearrange("(c p) b -> p c b", p=128)
            for bc_i in range(BC):
                engs[bc_i].dma_start(xik_sb[:, bc_i, :],
                                     xik_d[bc_i * 128:(bc_i + 1) * 128, :])
            nc.gpsimd.dma_start(wl_sb[:], wl_d.rearrange(
                "(c p) f -> p c f", p=128))
            nc.sync.dma_start(xt_sb[:, 0:4, :], xt3[:, 0:4, :])
            nc.scalar.dma_start(xt_sb[:, 4:NCH, :], xt3[:, 4:NCH, :])
            nc.sync.dma_start(xt8_sb[:], xt8_d.rearrange(
                "(c p) b -> p c b", p=128))
            nc.gpsimd.dma_start(f_sb[:], f_d[:])

            wl4 = wl_sb[:].rearrange("p c (n o) -> p c n o", n=N_NODE)

            # ---------------- helpers ----------------
            def s_matmul(rhs3, s_sb, scale):
                """s_sb[:,bc,:] = scale * sum_c xt[:,c,bc].T @ rhs3[:,c,:]"""
                for bc_i in range(BC):
                    s_ps = ps_s.tile([128, NO], f32, name="s_ps", tag="s_ps")
                    for c in range(NCH):
                        nc.tensor.matmul(
                            s_ps[:],
                            xt_sb[:, c, bc_i * 128:(bc_i + 1) * 128],
                            rhs3[:, c, :],
                            start=(c == 0), stop=(c == NCH - 1))
                    if scale is None:
                        nc.scalar.copy(s_sb[:, bc_i, :], s_ps[:])
                    else:
                        nc.scalar.mul(s_sb[:, bc_i, :], s_ps[:], scale)

            def allgather_s(s_sb, t):
                """AllGather the fp8 s partials (AG is cheaper than
                AllReduce on this stack) and tree-reduce the 8 rank partials
                on the DVE. Rounding only perturbs the routing weights."""
                ag_in = dp.tile([128, BC * NO], f8, name=f"ag_in{t}",
                                tag="ag_in")
                ag_out = dp.tile([N_CORES * 128, BC * NO], f8,
                                 name=f"ag_out{t}", tag="ag_out")
                for bc_i in range(BC):
                    engs[bc_i % 2].dma_start(
                        ag_in[:, bc_i * NO:(bc_i + 1) * NO],
                        s_sb[:, bc_i, :])
                nc.gpsimd.collective_compute(
                    "AllGather", ALU.bypass, replica_groups=RG,
                    ins=[ag_in.opt()], outs=[ag_out.opt()])
                ag3 = ag_out.rearrange("(r p) f -> p r f", p=128)
                nh = N_CORES // 2
                gengs = [nc.sync, nc.scalar, nc.gpsimd, nc.sync]
                # one SBUF tile per rank-pair so each leaf add depends only
                # on its own gather DMA, not on all four
                agvs = [wp.tile([128, 2, BC * NO], f8, name=f"agv{h}",
                                tag=f"agv{h}") for h in range(nh)]
                for h in range(nh):
                    gengs[h].dma_start(agvs[h][:],
                                       ag3[:, 2 * h:2 * h + 2, :])
                # leaf adds pair the two ranks of each DMA so the tree starts
                # as soon as individual transfers land
                t4 = wp.tile([128, nh, BC * NO], bf16, name="agt4",
                             tag="agt4")
                for h in range(nh):
                    nc.vector.tensor_add(t4[:, h, :], agvs[h][:, 0, :],
                                         agvs[h][:, 1, :])
                cur = t4[:]
                w = nh
                while w > 2:
                    w //= 2
                    nxt = wp.tile([128, w, BC * NO], bf16,
                                  name=f"agt{w}", tag=f"agt{w}")
                    nc.vector.tensor_add(nxt[:], cur[:, 0:w, :],
                                         cur[:, w:2 * w, :])
                    cur = nxt[:]
                sfull = wp.tile([128, BC, NO], bf16, name="sfull",
                                tag="sfull")
                nc.vector.tensor_add(
                    sfull[:].rearrange("p c f -> p (c f)"),
                    cur[:, 0, :], cur[:, 1, :])
                return sfull

            def rsqrt(msq, P, nch, tag, iters):
                """z ~ 1/sqrt(msq) via int bit-trick + Newton steps (DVE
                only -- avoids the Sqrt/Ln ACT table sets entirely)."""
                sh = [P, nch, N_NODE]
                zi = wp.tile(sh, i32, name="zi" + tag, tag="zi" + tag)
                nc.vector.tensor_scalar(
                    out=zi[:], in0=msq[:].bitcast(i32), scalar1=1, scalar2=-1,
                    op0=ALU.arith_shift_right, op1=ALU.bitwise_xor)
                nc.vector.tensor_scalar_add(zi[:], zi[:], RSQRT_MAGIC + 1)
                z = zi[:].bitcast(f32)
                if iters:
                    t = wp.tile(sh, f32, name="nt" + tag, tag="nt" + tag)
                    w = wp.tile(sh, f32, name="nw" + tag, tag="nw" + tag)
                for _ in range(iters):
                    nc.vector.tensor_mul(t[:], z, z)
                    nc.vector.tensor_mul(t[:], t[:], msq[:])
                    nc.vector.tensor_scalar(
                        out=w[:], in0=t[:], scalar1=-0.5, scalar2=1.5,
                        op0=ALU.mult, op1=ALU.add)
                    nc.vector.tensor_mul(z, z, w[:])
                return z

            def squash(s_sb, P, nch, tag, v_dtype, newton_iters=1):
                """v = squash(s) over o. s_sb [P, nch, NO]."""
                s4 = s_sb[:].rearrange("p c (n o) -> p c n o", n=N_NODE)
                sq = wp.tile([P, nch, NO], s_sb.dtype, name="sq" + tag,
                             tag="sq" + tag)
                nc.vector.tensor_mul(sq[:], s_sb[:], s_sb[:])
                msq = wp.tile([P, nch, N_NODE], f32, name="msq" + tag,
                              tag="msq" + tag)
                nc.vector.reduce_sum(
                    msq[:], sq[:].rearrange("p c (n o) -> p c n o", n=N_NODE),
                    axis=AX.X)
                z = rsqrt(msq, P, nch, tag, newton_iters)
                mag = wp.tile([P, nch, N_NODE], f32, name="mag" + tag,
                              tag="mag" + tag)
                nc.vector.tensor_mul(mag[:], msq[:], z)   # sqrt(msq)
                den = wp.tile([P, nch, N_NODE], f32, name="den" + tag,
                              tag="den" + tag)
                nc.vector.tensor_scalar_add(den[:], msq[:], 1.0)
                rden = wp.tile([P, nch, N_NODE], f32, name="rden" + tag,
                               tag="rden" + tag)
                nc.vector.reciprocal(rden[:], den[:])
                fac = wp.tile([P, nch, N_NODE], f32, name="fac" + tag,
                              tag="fac" + tag)
                nc.vector.tensor_mul(fac[:], mag[:], rden[:])
                v_sb = wp.tile([P, nch, NO], v_dtype, name="v" + tag,
                               tag="v" + tag)
                fb = fac[:].unsqueeze(3).broadcast_to((P, nch, N_NODE, O_SZ))
                nc.vector.tensor_mul(
                    v_sb[:].rearrange("p c (n o) -> p c n o", n=N_NODE),
                    s4, fb)
                return v_sb

            def b_update(v_sb, first):
                # Q matmuls pack 3 j-chunks per PSUM bank; p = wl * Q reads
                # each bank straight out of PSUM (3 wide TTs, no Q copies).
                # xik/v are fp8 in exactly the [128, 2 k-tiles, M/N] DoubleRow
                # layout, so one matmul contracts the whole batch of 256.
                DR = mybir.MatmulPerfMode.DoubleRow
                p_sb = wp.tile([128, NCH, NO], bf16, name="p_sb", tag="p_sb")
                pr = wp.tile([128, NCH, N_NODE], f32, name="pr_sb",
                             tag="pr_sb")
                q_banks = []
                for g in range(NCH // 3):
                    q_ps = ps_q.tile([128, 3 * NO], f32, name="q_ps",
                                     tag="q_ps")
                    q_banks.append(q_ps)
                    for s_i in range(3):
                        mc = g * 3 + s_i
                        nc.tensor.matmul(
                            q_ps[:, s_i * NO:(s_i + 1) * NO],
                            xik_sb[:, :, mc * 128:(mc + 1) * 128],
                            v_sb[:],
                            start=True, stop=True, perf_mode=DR)
                    # p reads Q straight out of PSUM, so it must stay on the
                    # DVE (gpsimd cannot access PSUM; free-axis reduce is
                    # DVE-only too)
                    nc.vector.tensor_mul(
                        p_sb[:, g * 3:(g + 1) * 3, :],
                        wl_sb[:, g * 3:(g + 1) * 3, :],
                        q_ps[:].rearrange("p (c f) -> p c f", c=3))
                for g in range(NCH // 3):
                    nc.vector.reduce_sum(
                        pr[:, g * 3:(g + 1) * 3, :],
                        p_sb[:, g * 3:(g + 1) * 3, :].rearrange(
                            "p c (n o) -> p c n o", n=N_NODE),
                        axis=AX.X)
                prb = wp.tile([128, NCH, N_NODE], bf16, name="prb",
                              tag="prb")
                nc.vector.tensor_copy(prb[:], pr[:])
                uv_ps = ps_f.tile([128, NCH * N_NODE], f32, name="uv_ps",
                                  tag="uv_ps")
                nc.tensor.matmul(uv_ps[:], f_sb[:],
                                 prb[:].rearrange("p c n -> p (c n)"),
                                 start=True, stop=True)
                uv3 = uv_ps[:].rearrange("p (c n) -> p c n", n=N_NODE)
                if first:
                    nc.scalar.copy(b_sb[:], uv3)
                    return uv3
                nc.vector.tensor_add(b_sb[:], b_sb[:], uv3)
                return b_sb[:]

            def softmax_mc(b_src, mc_dtype):
                e_sb = wp.tile([128, NCH, N_NODE], f32, name="e_sb",
                               tag="e_sb")
                nc.scalar.activation(e_sb[:], b_src, AF.Exp)
                se = wp.tile([128, NCH], f32, name="se", tag="se")
                nc.vector.reduce_sum(se[:], e_sb[:], axis=AX.X)
                rse = wp.tile([128, NCH], f32, name="rse", tag="rse")
                nc.vector.reciprocal(rse[:], se[:])
                c_sb = wp.tile([128, NCH, N_NODE], bf16, name="c_sb",
                               tag="c_sb")
                nc.vector.tensor_mul(
                    c_sb[:], e_sb[:],
                    rse[:].unsqueeze(2).broadcast_to((128, NCH, N_NODE)))
                mc_sb = wp.tile([128, NCH, NO], mc_dtype, name="mc_sb",
                                tag="mc_sb" + str(mc_dtype))
                cb = c_sb[:].unsqueeze(3).broadcast_to(
                    (128, NCH, N_NODE, O_SZ))
                mc4 = mc_sb[:].rearrange("p c (n o) -> p c n o", n=N_NODE)
                # split the W-sized multiply across DVE and the idle GpSimd
                # (gpsimd is ~4.5x slower per element -> 8/1 split balances)
                nc.vector.tensor_mul(mc4[:, 0:8], wl4[:, 0:8], cb[:, 0:8])
                nc.gpsimd.tensor_mul(mc4[:, 8:NCH], wl4[:, 8:NCH],
                                     cb[:, 8:NCH])
                return mc_sb

            # ------------- iteration 1 (c uniform = 0.1, replicated) -------
            sfull = wp.tile([128, BC, NO], f32, name="s1full", tag="s1full")
            for bc_i in range(BC):
                s_ps = ps_s.tile([128, NO], f32, name="s_ps", tag="s_ps")
                for c in range(NCF):
                    nc.tensor.matmul(
                        s_ps[:],
                        xtf_sb[:, c, bc_i * 128:(bc_i + 1) * 128],
                        wlf_sb[:, c, :],
                        start=(c == 0), stop=(c == NCF - 1))
                nc.scalar.mul(sfull[:, bc_i, :], s_ps[:], 0.1)
            v_sb = squash(sfull, 128, BC, "m", f8, newton_iters=0)
            b_src = b_update(v_sb, first=True)

            # ---------------- iteration 2 ----------------
            # s2 only steers routing: fp8 xt/mc with DoubleRow pairs two
            # 128-row k-tiles per matmul (4 DR + 1 plain f8 per batch chunk)
            mc_sb = softmax_mc(b_src, f8)
            s_sb = wp.tile([128, BC, NO], f8, name="s_sbr", tag="s_sbr")
            DR = mybir.MatmulPerfMode.DoubleRow
            for bc_i in range(BC):
                s_ps = ps_s.tile([128, NO], f32, name="s_ps", tag="s_ps")
                bsl = slice(bc_i * 128, (bc_i + 1) * 128)
                for cp in range(4):
                    nc.tensor.matmul(
                        s_ps[:], xt8_sb[:, 2 * cp:2 * cp + 2, bsl],
                        mc_sb[:, 2 * cp:2 * cp + 2, :],
                        start=(cp == 0), stop=False, perf_mode=DR)
                nc.tensor.matmul(s_ps[:], xt8_sb[:, 8, bsl],
                                 mc_sb[:, 8, :], start=False, stop=True)
                nc.scalar.copy(s_sb[:, bc_i, :], s_ps[:])
            sfull = allgather_s(s_sb, 1)
            v_sb = squash(sfull, 128, BC, "m", f8, newton_iters=0)
            b_src = b_update(v_sb, first=False)

            # ---------------- iteration 3 (no b-update) ----------------
            mc_sb = softmax_mc(b_src, bf16)
            s_sb = wp.tile([128, BC, NO], bf16, name="s_sb", tag="s_sb")
            s_matmul(mc_sb[:], s_sb, scale=None)

            # AllToAll instead of ReduceScatter: the RS mesh serializes ~5
            # per-hop waits (~12.5us) while A2A is a pure permutation. Batch
            # row-block j of my bf16 s3 partial goes to rank j; I get back
            # all 8 ranks' partials for MY 32 rows and reduce them on-chip.
            rs_in = dp.tile([B, NO], bf16, name="rs_in", tag="rs_in")
            rs_out = dp.tile([B, NO], bf16, name="rs_out", tag="rs_out")
            for bc_i in range(BC):
                engs[bc_i % 2].dma_start(
                    rs_in[bc_i * 128:(bc_i + 1) * 128, :], s_sb[:, bc_i, :])
            nc.gpsimd.collective_compute(
                "AllToAll", ALU.bypass, replica_groups=RG,
                ins=[rs_in.opt()], outs=[rs_out.opt()])
            a2a = wp.tile([B_SH, N_CORES, NO], bf16, name="a2a", tag="a2a")
            nc.sync.dma_start(
                a2a[:], rs_out.rearrange("(r p) f -> p r f", p=B_SH))
            s8 = wp.tile([B_SH, 4, NO], f32, name="s8", tag="s8")
            nc.vector.tensor_add(s8[:], a2a[:, 0:4, :], a2a[:, 4:8, :])
            s4 = wp.tile([B_SH, 2, NO], f32, name="s4", tag="s4")
            nc.vector.tensor_add(s4[:], s8[:, 0:2, :], s8[:, 2:4, :])
            ssh = wp.tile([B_SH, 1, NO], f32, name="ssh", tag="ssh")
            nc.vector.tensor_add(ssh[:], s4[:, 0:1, :], s4[:, 1:2, :])
            vsh = squash(ssh, B_SH, 1, "s", f32, newton_iters=1)
            nc.sync.dma_start(y_d[:], vsh[:, 0, :])

    nc.compile()
    return nc


def _host_prep(x, W):
    """Per-core input dicts + the constant F matrix."""
    import ml_dtypes

    bf = ml_dtypes.bfloat16
    x = np.ascontiguousarray(x, dtype=np.float32)
    W = np.ascontiguousarray(W, dtype=np.float32)
    F = (np.kron(np.eye(16, dtype=np.float32),
                 np.ones((8, 8), dtype=np.float32)) / np.float32(B)).astype(bf)
    xtF = np.ascontiguousarray(x.transpose(2, 1, 0)).reshape(JF, B).astype(bf)
    wlF = np.ascontiguousarray(
        (np.float32(0.03) * W[0]).transpose(0, 3, 1, 2)
    ).reshape(JF, NO).astype(bf)
    in_maps = []
    f8 = ml_dtypes.float8_e4m3fn
    for c in range(N_CORES):
        sl = slice(c * I_SH, (c + 1) * I_SH)
        rsl = slice(c * JR, (c + 1) * JR)
        xik = np.ascontiguousarray(
            x[:, :, sl].transpose(0, 2, 1)).reshape(B, JR).astype(f8)
        in_maps.append({
            "xtf": xtF, "wlf": wlF,
            "xt": xtF[rsl], "xt8": xtF[rsl].astype(f8), "wl": wlF[rsl],
            "xik": xik, "fmat": F,
        })
    return in_maps


def _run(in_maps, trace=False, all_cores=False):
    from concourse.bass_utils import run_bass_kernel_spmd

    if "nc" not in _CACHE:
        _CACHE["nc"] = _build_program()
    nc = _CACHE["nc"]
    kwargs = {}
    if all_cores:
        kwargs["trace_cores"] = list(range(N_CORES))
    res = run_bass_kernel_spmd(nc, in_maps, core_ids=list(range(N_CORES)),
                               trace=trace, **kwargs)
    return res


def kernel(x: np.ndarray, W: np.ndarray) -> np.ndarray:
    in_maps = _host_prep(x, W)
    res = _run(in_maps)
    v = np.concatenate([res.results[c]["y"] for c in range(N_CORES)], axis=0)
    return v.reshape(B, N_NODE, O_SZ, 1).astype(np.float32)


# revision 37
# speedup vs baseline: 1.0212x; 1.0212x over previous
"""CapsuleLayer (dynamic routing, 3 iterations) on 8 Trainium2 NeuronCores.

Structure — replicated iteration 1, i-sharded iterations 2-3:
  - Iteration 1 uses uniform c = 1/10 (softmax of zeros), so
    s1 = 0.1 * (xt_full.T @ Wl_full) needs no routing state. Instead of
    computing an i-shard partial and paying an AllGather round (the
    collective subsystem is still booting until ~80us anyway), EVERY core
    computes the FULL s1 itself: 72 chunk matmuls on the otherwise-idle
    PE, fed by a full (replicated) bf16 copy of xt and Wl (~8MB DMA).
    This removes the iteration-1 AllGather round trip (~35us) entirely.
  - Iterations 2-3 are i-sharded: 144 capsules per core, 1152 rows = 9
    chunks of 128. The iteration-2 s partials are exchanged via one
    fp8-e4m3 AllGather + on-chip tree reduce; the iteration-3 partials
    via one bf16 AllToAll (cheaper mesh than ReduceScatter), after which
    each core owns the 8 rank partials for its 32-row batch shard and
    sums them on the PE with S = kron(ones(4,1), I32).
  - b_ij update via the Gram trick: Q = xik.T @ v (PE), p = Wl*Q (DVE),
    uv = F.T @ p with F = kron(I16, ones8x8)/B (PE), keeping b
    row-replicated over k. xik and v are fp8 in the exact [128p, 2 kt,
    M/N] DoubleRow layout, so one matmul contracts the whole batch.
  - A tiny warm-up AllGather at kernel start overlaps the one-time
    ncfw/collective boot (~70us) with input DMA + the s1 compute. The
    boot (plus cross-core boot skew absorbed by the first real mesh) is
    the dominant fixed cost of the kernel.
  - sqrt via int bit-trick (+1 Newton step on the output path only) on
    the DVE — avoids the Sqrt/Ln ACT table loads entirely.
  - Precision plan: everything that only steers routing (s1/s2, v1/v2,
    Q, c) runs in bf16/fp8; the output-determining s3 matmul and final
    squash run bf16/f32. Measured ~4e-3 vs the 2e-2 gate.
"""
import sys

if "/opt/trn_rl_repo" not in sys.path:
    sys.path.insert(0, "/opt/trn_rl_repo")

import numpy as np

import os
N_CORES = int(os.environ.get("KERNEL_CORES", "8"))
B, IN_SIZE, I_TOT = 256, 8, 1152
N_NODE, O_SZ = 10, 16
NO = N_NODE * O_SZ          # 160
I_SH = I_TOT // N_CORES     # 144 capsules per core
JR = I_SH * IN_SIZE         # 1152 local rows per core
NCH = JR // 128             # 9 local contraction chunks
JF = I_TOT * IN_SIZE        # 9216 full rows
NCF = JF // 128             # 72 full contraction chunks
BC = B // 128               # 2 batch chunks
B_SH = B // N_CORES         # 32 batch rows per core after ReduceScatter

RSQRT_MAGIC = 0x5F3759DF

_CACHE = {}


def _build_program():
    import concourse.bacc as bacc
    import concourse.tile as tile
    import concourse.mybir as mybir

    f32 = mybir.dt.float32
    bf16 = mybir.dt.bfloat16
    f8 = mybir.dt.float8e4
    i32 = mybir.dt.int32
    AF = mybir.ActivationFunctionType
    ALU = mybir.AluOpType
    AX = mybir.AxisListType

    nc = bacc.Bacc("TRN2", target_bir_lowering=False, debug=False,
                   enable_asserts=True, num_devices=N_CORES)

    xtf_d = nc.dram_tensor("xtf", [JF, B], bf16, kind="ExternalInput").ap()
    wlf_d = nc.dram_tensor("wlf", [JF, NO], bf16, kind="ExternalInput").ap()
    xt_d = nc.dram_tensor("xt", [JR, B], bf16, kind="ExternalInput").ap()
    xt8_d = nc.dram_tensor("xt8", [JR, B], f8, kind="ExternalInput").ap()
    xik_d = nc.dram_tensor("xik", [B, JR], f8, kind="ExternalInput").ap()
    wl_d = nc.dram_tensor("wl", [JR, NO], bf16, kind="ExternalInput").ap()
    f_d = nc.dram_tensor("fmat", [128, 128], bf16, kind="ExternalInput").ap()
    s_d = nc.dram_tensor("smat", [128, B_SH], bf16, kind="ExternalInput").ap()
    y_d = nc.dram_tensor("y", [B_SH, NO], f32, kind="ExternalOutput").ap()

    RG = [list(range(N_CORES))]

    with tile.TileContext(nc) as tc:
        with tc.tile_pool(name="persist", bufs=1) as pp, \
             tc.tile_pool(name="work", bufs=1) as wp, \
             tc.tile_pool(name="ps_s", bufs=2, space="PSUM") as ps_s, \
             tc.tile_pool(name="ps_q", bufs=3, space="PSUM") as ps_q, \
             tc.tile_pool(name="ps_f", bufs=1, space="PSUM") as ps_f, \
             tc.tile_pool(name="dram", bufs=1, space="DRAM") as dp:

            # ---------------- input loads ----------------
            xtf_sb = pp.tile([128, NCF, B], bf16, name="xtf_sb", tag="xtf_sb")
            wlf_sb = pp.tile([128, NCF, NO], bf16, name="wlf_sb",
                             tag="wlf_sb")
            xt_sb = pp.tile([128, NCH, B], bf16, name="xt_sb", tag="xt_sb")
            xt8_sb = pp.tile([128, NCH, B], f8, name="xt8_sb", tag="xt8_sb")
            xik_sb = pp.tile([128, BC, JR], f8, name="xik_sb", tag="xik_sb")
            wl_sb = pp.tile([128, NCH, NO], bf16, name="wl_sb", tag="wl_sb")
            f_sb = pp.tile([128, 128], bf16, name="f_sb", tag="f_sb")
            smat_sb = pp.tile([128, B_SH], bf16, name="smat_sb",
                              tag="smat_sb")
            b_sb = pp.tile([128, NCH, N_NODE], f32, name="b_sb", tag="b_sb")

            # Warm-up collective: starts the one-time ncfw/TOPSP collective
            # boot (~68us) at program start, overlapping the input DMAs and
            # the replicated s1 compute.
            warm_in = dp.tile([128, 4], bf16, name="warm_in", tag="warm_in")
            warm_out = dp.tile([N_CORES * 128, 4], bf16, name="warm_out",
                               tag="warm_out")
            nc.gpsimd.collective_compute(
                "AllGather", ALU.bypass, replica_groups=RG,
                ins=[warm_in.opt()], outs=[warm_out.opt()])

            # Full xt/Wl stream in 12-chunk groups round-robined over the
            # three DMA-capable engine queues, in s1 consumption order.
            engs = [nc.sync, nc.scalar, nc.gpsimd]
            xtf3 = xtf_d.rearrange("(c p) b -> p c b", p=128)
            wlf3 = wlf_d.rearrange("(c p) f -> p c f", p=128)
            GW = 12
            for g in range(NCF // GW):
                eng = engs[g % 3]
                sl = slice(g * GW, (g + 1) * GW)
                eng.dma_start(xtf_sb[:, sl, :], xtf3[:, sl, :])
                eng.dma_start(wlf_sb[:, sl, :], wlf3[:, sl, :])
            # Local tensors (needed from the iter-1 b-update onward).
            xt3 = xt_d.rearrange("(c p) b -> p c b", p=128)
            for bc_i in range(BC):
                engs[bc_i].dma_start(xik_sb[:, bc_i, :],
                                     xik_d[bc_i * 128:(bc_i + 1) * 128, :])
            nc.gpsimd.dma_start(wl_sb[:], wl_d.rearrange(
                "(c p) f -> p c f", p=128))
            nc.sync.dma_start(xt_sb[:, 0:4, :], xt3[:, 0:4, :])
            nc.scalar.dma_start(xt_sb[:, 4:NCH, :], xt3[:, 4:NCH, :])
            nc.sync.dma_start(xt8_sb[:], xt8_d.rearrange(
                "(c p) b -> p c b", p=128))
            nc.gpsimd.dma_start(f_sb[:], f_d[:])
            nc.gpsimd.dma_start(smat_sb[:], s_d[:])

            wl4 = wl_sb[:].rearrange("p c (n o) -> p c n o", n=N_NODE)

            # ---------------- helpers ----------------
            def s_matmul(rhs3, s_sb, scale):
                """s_sb[:,bc,:] = scale * sum_c xt[:,c,bc].T @ rhs3[:,c,:]"""
                for bc_i in range(BC):
                    s_ps = ps_s.tile([128, NO], f32, name="s_ps", tag="s_ps")
                    for c in range(NCH):
                        nc.tensor.matmul(
                            s_ps[:],
                            xt_sb[:, c, bc_i * 128:(bc_i + 1) * 128],
                            rhs3[:, c, :],
                            start=(c == 0), stop=(c == NCH - 1))
                    if scale is None:
                        nc.scalar.copy(s_sb[:, bc_i, :], s_ps[:])
                    else:
                        nc.scalar.mul(s_sb[:, bc_i, :], s_ps[:], scale)

            def allgather_s(s_sb, t):
                """AllGather the fp8 s partials (AG is cheaper than
                AllReduce on this stack) and tree-reduce the 8 rank partials
                on the DVE. Rounding only perturbs the routing weights."""
                ag_in = dp.tile([128, BC * NO], f8, name=f"ag_in{t}",
                                tag="ag_in")
                ag_out = dp.tile([N_CORES * 128, BC * NO], f8,
                                 name=f"ag_out{t}", tag="ag_out")
                for bc_i in range(BC):
                    engs[bc_i % 2].dma_start(
                        ag_in[:, bc_i * NO:(bc_i + 1) * NO],
                        s_sb[:, bc_i, :])
                nc.gpsimd.collective_compute(
                    "AllGather", ALU.bypass, replica_groups=RG,
                    ins=[ag_in.opt()], outs=[ag_out.opt()])
                ag3 = ag_out.rearrange("(r p) f -> p r f", p=128)
                nh = N_CORES // 2
                gengs = [nc.sync, nc.scalar, nc.gpsimd, nc.sync]
                # one SBUF tile per rank-pair so each leaf add depends only
                # on its own gather DMA, not on all four
                agvs = [wp.tile([128, 2, BC * NO], f8, name=f"agv{h}",
                                tag=f"agv{h}") for h in range(nh)]
                for h in range(nh):
                    gengs[h].dma_start(agvs[h][:],
                                       ag3[:, 2 * h:2 * h + 2, :])
                # leaf adds pair the two ranks of each DMA so the tree starts
                # as soon as individual transfers land
                t4 = wp.tile([128, nh, BC * NO], bf16, name="agt4",
                             tag="agt4")
                for h in range(nh):
                    nc.vector.tensor_add(t4[:, h, :], agvs[h][:, 0, :],
                                         agvs[h][:, 1, :])
                cur = t4[:]
                w = nh
                while w > 2:
                    w //= 2
                    nxt = wp.tile([128, w, BC * NO], bf16,
                                  name=f"agt{w}", tag=f"agt{w}")
                    nc.vector.tensor_add(nxt[:], cur[:, 0:w, :],
                                         cur[:, w:2 * w, :])
                    cur = nxt[:]
                sfull = wp.tile([128, BC, NO], bf16, name="sfull",
                                tag="sfull")
                nc.vector.tensor_add(
                    sfull[:].rearrange("p c f -> p (c f)"),
                    cur[:, 0, :], cur[:, 1, :])
                return sfull

            def rsqrt(msq, P, nch, tag, iters):
                """z ~ 1/sqrt(msq) via int bit-trick + Newton steps (DVE
                only -- avoids the Sqrt/Ln ACT table sets entirely)."""
                sh = [P, nch, N_NODE]
                zi = wp.tile(sh, i32, name="zi" + tag, tag="zi" + tag)
                nc.vector.tensor_scalar(
                    out=zi[:], in0=msq[:].bitcast(i32), scalar1=1, scalar2=-1,
                    op0=ALU.arith_shift_right, op1=ALU.bitwise_xor)
                nc.vector.tensor_scalar_add(zi[:], zi[:], RSQRT_MAGIC + 1)
                z = zi[:].bitcast(f32)
                if iters:
                    t = wp.tile(sh, f32, name="nt" + tag, tag="nt" + tag)
                    w = wp.tile(sh, f32, name="nw" + tag, tag="nw" + tag)
                for _ in range(iters):
                    nc.vector.tensor_mul(t[:], z, z)
                    nc.vector.tensor_mul(t[:], t[:], msq[:])
                    nc.vector.tensor_scalar(
                        out=w[:], in0=t[:], scalar1=-0.5, scalar2=1.5,
                        op0=ALU.mult, op1=ALU.add)
                    nc.vector.tensor_mul(z, z, w[:])
                return z

            def squash(s_sb, P, nch, tag, v_dtype, newton_iters=1):
                """v = squash(s) over o. s_sb [P, nch, NO]."""
                s4 = s_sb[:].rearrange("p c (n o) -> p c n o", n=N_NODE)
                sq = wp.tile([P, nch, NO], s_sb.dtype, name="sq" + tag,
                             tag="sq" + tag)
                nc.vector.tensor_mul(sq[:], s_sb[:], s_sb[:])
                msq = wp.tile([P, nch, N_NODE], f32, name="msq" + tag,
                              tag="msq" + tag)
                nc.vector.reduce_sum(
                    msq[:], sq[:].rearrange("p c (n o) -> p c n o", n=N_NODE),
                    axis=AX.X)
                z = rsqrt(msq, P, nch, tag, newton_iters)
                mag = wp.tile([P, nch, N_NODE], f32, name="mag" + tag,
                              tag="mag" + tag)
                nc.vector.tensor_mul(mag[:], msq[:], z)   # sqrt(msq)
                den = wp.tile([P, nch, N_NODE], f32, name="den" + tag,
                              tag="den" + tag)
                nc.vector.tensor_scalar_add(den[:], msq[:], 1.0)
                rden = wp.tile([P, nch, N_NODE], f32, name="rden" + tag,
                               tag="rden" + tag)
                nc.vector.reciprocal(rden[:], den[:])
                fac = wp.tile([P, nch, N_NODE], f32, name="fac" + tag,
                              tag="fac" + tag)
                nc.vector.tensor_mul(fac[:], mag[:], rden[:])
                v_sb = wp.tile([P, nch, NO], v_dtype, name="v" + tag,
                               tag="v" + tag)
                fb = fac[:].unsqueeze(3).broadcast_to((P, nch, N_NODE, O_SZ))
                nc.vector.tensor_mul(
                    v_sb[:].rearrange("p c (n o) -> p c n o", n=N_NODE),
                    s4, fb)
                return v_sb

            def b_update(v_sb, first):
                # Q matmuls pack 3 j-chunks per PSUM bank; p = wl * Q reads
                # each bank straight out of PSUM (3 wide TTs, no Q copies).
                # xik/v are fp8 in exactly the [128, 2 k-tiles, M/N] DoubleRow
                # layout, so one matmul contracts the whole batch of 256.
                DR = mybir.MatmulPerfMode.DoubleRow
                p_sb = wp.tile([128, NCH, NO], bf16, name="p_sb", tag="p_sb")
                pr = wp.tile([128, NCH, N_NODE], f32, name="pr_sb",
                             tag="pr_sb")
                for g in range(NCH // 3):
                    q_ps = ps_q.tile([128, 3 * NO], f32, name="q_ps",
                                     tag="q_ps")
                    for s_i in range(3):
                        mc = g * 3 + s_i
                        nc.tensor.matmul(
                            q_ps[:, s_i * NO:(s_i + 1) * NO],
                            xik_sb[:, :, mc * 128:(mc + 1) * 128],
                            v_sb[:],
                            start=True, stop=True, perf_mode=DR)
                    # p reads Q straight out of PSUM, so it must stay on the
                    # DVE (gpsimd cannot access PSUM; free-axis reduce is
                    # DVE-only too)
                    nc.vector.tensor_mul(
                        p_sb[:, g * 3:(g + 1) * 3, :],
                        wl_sb[:, g * 3:(g + 1) * 3, :],
                        q_ps[:].rearrange("p (c f) -> p c f", c=3))
                for g in range(NCH // 3):
                    nc.vector.reduce_sum(
                        pr[:, g * 3:(g + 1) * 3, :],
                        p_sb[:, g * 3:(g + 1) * 3, :].rearrange(
                            "p c (n o) -> p c n o", n=N_NODE),
                        axis=AX.X)
                prb = wp.tile([128, NCH, N_NODE], bf16, name="prb",
                              tag="prb")
                nc.vector.tensor_copy(prb[:], pr[:])
                uv_ps = ps_f.tile([128, NCH * N_NODE], f32, name="uv_ps",
                                  tag="uv_ps")
                nc.tensor.matmul(uv_ps[:], f_sb[:],
                                 prb[:].rearrange("p c n -> p (c n)"),
                                 start=True, stop=True)
                uv3 = uv_ps[:].rearrange("p (c n) -> p c n", n=N_NODE)
                if first:
                    nc.scalar.copy(b_sb[:], uv3)
                    return uv3
                nc.vector.tensor_add(b_sb[:], b_sb[:], uv3)
                return b_sb[:]

            def softmax_mc(b_src, mc_dtype):
                e_sb = wp.tile([128, NCH, N_NODE], f32, name="e_sb",
                               tag="e_sb")
                nc.scalar.activation(e_sb[:], b_src, AF.Exp)
                se = wp.tile([128, NCH], f32, name="se", tag="se")
                nc.vector.reduce_sum(se[:], e_sb[:], axis=AX.X)
                rse = wp.tile([128, NCH], f32, name="rse", tag="rse")
                nc.vector.reciprocal(rse[:], se[:])
                c_sb = wp.tile([128, NCH, N_NODE], bf16, name="c_sb",
                               tag="c_sb")
                nc.vector.tensor_mul(
                    c_sb[:], e_sb[:],
                    rse[:].unsqueeze(2).broadcast_to((128, NCH, N_NODE)))
                mc_sb = wp.tile([128, NCH, NO], mc_dtype, name="mc_sb",
                                tag="mc_sb" + str(mc_dtype))
                cb = c_sb[:].unsqueeze(3).broadcast_to(
                    (128, NCH, N_NODE, O_SZ))
                mc4 = mc_sb[:].rearrange("p c (n o) -> p c n o", n=N_NODE)
                # split the W-sized multiply across DVE and the idle GpSimd
                # (gpsimd is ~4.5x slower per element -> 8/1 split balances)
                nc.vector.tensor_mul(mc4[:, 0:8], wl4[:, 0:8], cb[:, 0:8])
                nc.gpsimd.tensor_mul(mc4[:, 8:NCH], wl4[:, 8:NCH],
                                     cb[:, 8:NCH])
                return mc_sb

            # ------------- iteration 1 (c uniform = 0.1, replicated) -------
            sfull = wp.tile([128, BC, NO], f32, name="s1full", tag="s1full")
            for bc_i in range(BC):
                s_ps = ps_s.tile([128, NO], f32, name="s_ps", tag="s_ps")
                for c in range(NCF):
                    nc.tensor.matmul(
                        s_ps[:],
                        xtf_sb[:, c, bc_i * 128:(bc_i + 1) * 128],
                        wlf_sb[:, c, :],
                        start=(c == 0), stop=(c == NCF - 1))
                nc.scalar.mul(sfull[:, bc_i, :], s_ps[:], 0.1)
            v_sb = squash(sfull, 128, BC, "m", f8, newton_iters=0)
            b_src = b_update(v_sb, first=True)

            # ---------------- iteration 2 ----------------
            # s2 only steers routing: fp8 xt/mc with DoubleRow pairs two
            # 128-row k-tiles per matmul (4 DR + 1 plain f8 per batch chunk)
            mc_sb = softmax_mc(b_src, f8)
            s_sb = wp.tile([128, BC, NO], f8, name="s_sbr", tag="s_sbr")
            DR = mybir.MatmulPerfMode.DoubleRow
            for bc_i in range(BC):
                s_ps = ps_s.tile([128, NO], f32, name="s_ps", tag="s_ps")
                bsl = slice(bc_i * 128, (bc_i + 1) * 128)
                for cp in range(4):
                    nc.tensor.matmul(
                        s_ps[:], xt8_sb[:, 2 * cp:2 * cp + 2, bsl],
                        mc_sb[:, 2 * cp:2 * cp + 2, :],
                        start=(cp == 0), stop=False, perf_mode=DR)
                nc.tensor.matmul(s_ps[:], xt8_sb[:, 8, bsl],
                                 mc_sb[:, 8, :], start=False, stop=True)
                nc.scalar.copy(s_sb[:, bc_i, :], s_ps[:])
            sfull = allgather_s(s_sb, 1)
            v_sb = squash(sfull, 128, BC, "m", f8, newton_iters=0)
            b_src = b_update(v_sb, first=False)

            # ---------------- iteration 3 (no b-update) ----------------
            mc_sb = softmax_mc(b_src, bf16)
            s_sb = wp.tile([128, BC, NO], bf16, name="s_sb", tag="s_sb")
            s_matmul(mc_sb[:], s_sb, scale=None)

            # AllToAll instead of ReduceScatter: the RS mesh serializes ~5
            # per-hop waits (~12.5us) while A2A is a pure permutation. Batch
            # row-block j of my bf16 s3 partial goes to rank j; I get back
            # all 8 ranks' partials for MY 32 rows and reduce them on-chip.
            rs_in = dp.tile([B, NO], bf16, name="rs_in", tag="rs_in")
            rs_out = dp.tile([B, NO], bf16, name="rs_out", tag="rs_out")
            for bc_i in range(BC):
                engs[bc_i % 2].dma_start(
                    rs_in[bc_i * 128:(bc_i + 1) * 128, :], s_sb[:, bc_i, :])
            nc.gpsimd.collective_compute(
                "AllToAll", ALU.bypass, replica_groups=RG,
                ins=[rs_in.opt()], outs=[rs_out.opt()])
            # Fold the 8 rank-blocks onto 128 partitions (ranks 0-3 -> free
            # chunk 0, ranks 4-7 -> chunk 1) and let the PE sum the 4 ranks
            # per chunk via S = kron(ones(4,1), I32); one DVE add merges the
            # two chunks. Much faster than serial wide adds on 32 partitions.
            a2a = wp.tile([128, BC, NO], bf16, name="a2a", tag="a2a")
            nc.sync.dma_start(
                a2a[:], rs_out.rearrange("(c p) f -> p c f", p=128))
            sps = ps_f.tile([B_SH, NO], f32, name="sps", tag="sps")
            for bc_i in range(BC):
                nc.tensor.matmul(sps[:], smat_sb[:], a2a[:, bc_i, :],
                                 start=(bc_i == 0), stop=(bc_i == BC - 1))
            ssh = wp.tile([B_SH, 1, NO], f32, name="ssh", tag="ssh")
            nc.scalar.copy(ssh[:, 0, :], sps[:])
            vsh = squash(ssh, B_SH, 1, "s", f32, newton_iters=1)
            nc.sync.dma_start(y_d[:], vsh[:, 0, :])

    nc.compile()
    return nc


def _host_prep(x, W):
    """Per-core input dicts + the constant F matrix."""
    import ml_dtypes

    bf = ml_dtypes.bfloat16
    x = np.ascontiguousarray(x, dtype=np.float32)
    W = np.ascontiguousarray(W, dtype=np.float32)
    F = (np.kron(np.eye(16, dtype=np.float32),
                 np.ones((8, 8), dtype=np.float32)) / np.float32(B)).astype(bf)
    S = np.tile(np.eye(B_SH, dtype=np.float32),
                (N_CORES // 2, 1)).astype(bf)
    xtF = np.ascontiguousarray(x.transpose(2, 1, 0)).reshape(JF, B).astype(bf)
    wlF = np.ascontiguousarray(
        (np.float32(0.03) * W[0]).transpose(0, 3, 1, 2)
    ).reshape(JF, NO).astype(bf)
    in_maps = []
    f8 = ml_dtypes.float8_e4m3fn
    for c in range(N_CORES):
        sl = slice(c * I_SH, (c + 1) * I_SH)
        rsl = slice(c * JR, (c + 1) * JR)
        xik = np.ascontiguousarray(
            x[:, :, sl].transpose(0, 2, 1)).reshape(B, JR).astype(f8)
        in_maps.append({
            "xtf": xtF, "wlf": wlF,
            "xt": xtF[rsl], "xt8": xtF[rsl].astype(f8), "wl": wlF[rsl],
            "xik": xik, "fmat": F, "smat": S,
        })
    return in_maps


def _run(in_maps, trace=False, all_cores=False):
    from concourse.bass_utils import run_bass_kernel_spmd

    if "nc" not in _CACHE:
        _CACHE["nc"] = _build_program()
    nc = _CACHE["nc"]
    kwargs = {}
    if all_cores:
        kwargs["trace_cores"] = list(range(N_CORES))
    res = run_bass_kernel_spmd(nc, in_maps, core_ids=list(range(N_CORES)),
                               trace=trace, **kwargs)
    return res


def kernel(x: np.ndarray, W: np.ndarray) -> np.ndarray:
    in_maps = _host_prep(x, W)
    res = _run(in_maps)
    v = np.concatenate([res.results[c]["y"] for c in range(N_CORES)], axis=0)
    return v.reshape(B, N_NODE, O_SZ, 1).astype(np.float32)
